# revision 35
# baseline (speedup 1.0000x reference)
"""Two-level additive attention pooling on 8 TRN2 NeuronCores.

Reference computation (G=1024 groups, N=512 set size, IN=256, H=128, O=128):
  x       = tanh(feat @ hq_w.T + hq_b)            [G,N,H]
  w1      = softmax(x @ hk_w.T + hk_b, axis=N)    (hk_b drops: softmax-invariant)
  stacked = sum_n w1 * x                          [G,H]
  y       = tanh(stacked @ mq_w.T + mq_b)         [G,H]
  w2      = softmax(y @ mk_w.T + mk_b, axis=G)    (mk_b drops)
  final   = sum_g w2 * y                          [H]
  out     = final @ out_w.T + out_b               [O]

Distribution: groups sharded 128/core across 8 cores (feat is 512 MB, the
dominant HBM traffic; the steady state is DMA-bound at ~330 GB/s/core).
Level-1 produces stackedT [H, 128] (f32) per core, AllGather'd in TWO halves
(32 KB/rank each) so the first half's level-2 matmul+tanh+exp+weighted-sum
runs overlapped with the second half of the feat stream; only the second
half's chain plus the tiny softmax combine sits in the kernel tail.

Level-1 layout ("layout A"): x kept transposed, xT [H=128 part, N=512 free];
feat is host-pre-transposed to [G, IN, N] so the contraction dim IN lands on
partitions.  feat streams from HBM as f32 (full memory-roofline traffic) and
is cast to bf16 in-flight by the SWDGE DMA — the PE runs bf16 matmuls (fp32
matmul measured ~4x slower).  Scores for 4 groups land on partitions
{0,32,64,96} of one PSUM tile via col-tiled M=32 matmuls so a single exp
covers 4 groups; exp's accum_out yields the softmax denominators for free.
The per-group weighted sum is one fused DVE scalar_tensor_tensor (multiply +
free-dim reduce) against a PE ones-broadcast of e.
"""

import numpy as np

import concourse.bass as bass
import concourse.bacc as bacc
import concourse.tile as tile
from concourse import mybir
from concourse.bass_utils import run_bass_kernel_spmd

F32 = mybir.dt.float32
BF16 = mybir.dt.bfloat16
AF = mybir.ActivationFunctionType
ALU = mybir.AluOpType
AX = mybir.AxisListType

N_CORES = 8
G, N, IN_DIM, HID, OUT_DIM = 1024, 512, 256, 128, 128
G_LOC = G // N_CORES          # 128 groups per core
KC = IN_DIM // 128            # 2 contraction chunks
GB = 4                        # groups per block (batched exp / DMA)
P = 128


def build_bass(g_loc: int = G_LOC) -> bacc.Bacc:
    G_LOC = g_loc  # noqa: N806 — local override for sim-sized builds
    n_blocks = G_LOC // GB
    HGL = G_LOC // 2              # groups per collective half
    nc = bacc.Bacc("TRN2", target_bir_lowering=False, debug=False,
                   num_devices=N_CORES)

    featT = nc.dram_tensor("featT", [G_LOC, IN_DIM, N], F32, kind="ExternalInput")
    hq_wT = nc.dram_tensor("hq_wT", [IN_DIM, HID], F32, kind="ExternalInput")
    hq_b = nc.dram_tensor("hq_b", [HID, 1], F32, kind="ExternalInput")
    hk_w = nc.dram_tensor("hk_w", [HID, 1], F32, kind="ExternalInput")
    mq_wT = nc.dram_tensor("mq_wT", [HID, HID], F32, kind="ExternalInput")
    mq_b = nc.dram_tensor("mq_b", [HID, 1], F32, kind="ExternalInput")
    mk_w = nc.dram_tensor("mk_w", [HID, 1], F32, kind="ExternalInput")
    out_wT = nc.dram_tensor("out_wT", [HID, OUT_DIM], F32, kind="ExternalInput")
    out_b = nc.dram_tensor("out_b", [OUT_DIM, 1], F32, kind="ExternalInput")
    ident = nc.dram_tensor("ident", [P, P], F32, kind="ExternalInput")
    out = nc.dram_tensor("out", [OUT_DIM, 1], F32, kind="ExternalOutput")

    rg = [list(range(N_CORES))]

    with tile.TileContext(nc) as tc:
        with (
            tc.tile_pool(name="consts", bufs=1) as consts,
            tc.tile_pool(name="accum", bufs=1) as accum,
            tc.tile_pool(name="dram", bufs=1, space="DRAM") as dram,
            tc.tile_pool(name="l2sb", bufs=1) as l2sb,
            tc.tile_pool(name="featp", bufs=8) as featp,
            tc.tile_pool(name="xp", bufs=3) as xp,
            tc.tile_pool(name="ep", bufs=3) as ep,
            tc.tile_pool(name="zp", bufs=3) as zp,
            tc.tile_pool(name="scratchp", bufs=3) as scratchp,
            tc.tile_pool(name="ps_x", bufs=3, space="PSUM") as ps_x,
            tc.tile_pool(name="ps_s", bufs=2, space="PSUM") as ps_s,
            tc.tile_pool(name="ps_e", bufs=2, space="PSUM") as ps_e,
            tc.tile_pool(name="ps_t", bufs=1, space="PSUM") as ps_t,
        ):
            # bf16 weight copies (cast in-flight by SWDGE)
            hq_wT_sb = consts.tile([P, KC, HID], BF16)
            for c in range(KC):
                nc.gpsimd.dma_start(out=hq_wT_sb[:, c, :],
                                    in_=hq_wT[c * 128:(c + 1) * 128, :])
            hk_w_sb = consts.tile([P, 1], BF16)
            nc.gpsimd.dma_start(out=hk_w_sb, in_=hk_w[:, :])
            mq_wT_sb = consts.tile([P, HID], F32)
            nc.sync.dma_start(out=mq_wT_sb, in_=mq_wT[:, :])
            mk_w_sb = consts.tile([P, 1], BF16)
            nc.gpsimd.dma_start(out=mk_w_sb, in_=mk_w[:, :])
            # f32 biases / final weights / identity (host-provided: building
            # it with gpsimd.affine_select would stall SWDGE feat DMA issue)
            hq_b_sb = consts.tile([P, 1], F32)
            nc.sync.dma_start(out=hq_b_sb, in_=hq_b[:, :])
            mq_b_sb = consts.tile([P, 1], F32)
            nc.sync.dma_start(out=mq_b_sb, in_=mq_b[:, :])
            out_wT_sb = consts.tile([P, OUT_DIM], F32)
            nc.sync.dma_start(out=out_wT_sb, in_=out_wT[:, :])
            out_b_sb = consts.tile([P, 1], F32)
            nc.sync.dma_start(out=out_b_sb, in_=out_b[:, :])
            ident128_f = consts.tile([P, P], F32)
            nc.sync.dma_start(out=ident128_f, in_=ident[:, :])
            ident128 = consts.tile([P, P], BF16)
            nc.vector.tensor_copy(ident128, ident128_f)
            ones_bf = consts.tile([P, P], BF16)
            nc.vector.memset(ones_bf, 1.0)
            ones_f32 = consts.tile([1, P], F32)
            nc.vector.memset(ones_f32, 1.0)
            # hk_w replicated across 32 columns: the scores matmul runs M=32
            # (same cost as M=1, free-dim bound) so it fills a whole 32-row
            # PSUM strip — keeps the batched exp's input fully initialized.
            hk_w32 = consts.tile([P, 32], BF16)
            nc.vector.tensor_copy(hk_w32, hk_w_sb[:, 0:1].broadcast_to((P, 32)))

            uT_sb = accum.tile([P, G_LOC], F32)        # unnormalized stackedT
            z_row = accum.tile([1, G_LOC], F32)        # per-group softmax denom
            stackedT_sb = accum.tile([P, G_LOC], F32)  # normalized

            # level-2 state (filled per collective half)
            yt = l2sb.tile([P, 2, N_CORES * HGL], BF16)
            z2p = l2sb.tile([1, 2], F32)
            fcols = [l2sb.tile([P, 1], F32, tag=f"fcol{i}", name=f"fcol{i}")
                     for i in range(2)]
            cc_in = [dram.tile([P, HGL], F32, name=f"cc_in{h}", tag=f"cc_in{h}")
                     for h in range(2)]
            cc_out = [dram.tile([N_CORES * P, HGL], F32, addr_space="Shared",
                                name=f"cc_out{h}", tag=f"cc_out{h}")
                      for h in range(2)]

            def start_half(h):
                """Normalize half h of stackedT and trigger its AllGather
                (cheap; runs inline with the feat stream)."""
                lo, hi = h * HGL, (h + 1) * HGL
                inv_z = zp.tile([1, HGL], F32, tag="inv_z", name="inv_z")
                nc.vector.reciprocal(inv_z, z_row[0:1, lo:hi])
                izbc = ps_e.tile([P, HGL], F32, tag="ebc", name="izbc")
                nc.tensor.matmul(izbc, ones_f32, inv_z, start=True, stop=True)
                nc.vector.tensor_mul(stackedT_sb[:, lo:hi],
                                     uT_sb[:, lo:hi], izbc)
                nc.sync.dma_start(out=cc_in[h][:, :],
                                  in_=stackedT_sb[:, lo:hi])
                nc.gpsimd.collective_compute(
                    "AllGather", ALU.bypass, replica_groups=rg,
                    ins=[cc_in[h][:, :].opt()], outs=[cc_out[h][:, :].opt()])

            def drain_half(h):
                """Consume half h's gathered stackedT (level-2 chain).  Runs
                under low priority so its collective-wait never sits in front
                of level-1 work in the in-order engine queues."""
                st_h = l2sb.tile([P, N_CORES, HGL], F32, tag=f"st{h}",
                                 name=f"st{h}")
                nc.sync.dma_start(
                    out=st_h,
                    in_=cc_out[h][:, :].rearrange("(r p) g -> p r g", p=128))
                CW = N_CORES * HGL
                yps = ps_x.tile([P, CW], F32, tag="xps", name="yps")
                for r in range(N_CORES):
                    nc.tensor.matmul(yps[:, r * HGL:(r + 1) * HGL],
                                     mq_wT_sb, st_h[:, r, :],
                                     start=True, stop=True)
                nc.scalar.activation(yt[:, h, :], yps, AF.Tanh, bias=mq_b_sb)
                s2ps = ps_t.tile([1, CW], F32, tag="zt4", name="s2ps")
                nc.tensor.matmul(s2ps, mk_w_sb, yt[:, h, :],
                                 start=True, stop=True)
                e2 = ep.tile([1, CW], BF16, tag="e2", name="e2")
                nc.scalar.activation(e2, s2ps, AF.Exp,
                                     accum_out=z2p[0:1, h:h + 1])
                e2bc = ps_e.tile([P, CW], F32, tag="ebc", name="e2bc")
                nc.tensor.matmul(e2bc, ones_bf[0:1, :], e2,
                                 start=True, stop=True)
                scr2 = l2sb.tile([P, CW], BF16, tag=f"scr2_{h}",
                                 name=f"scr2_{h}")
                nc.vector.scalar_tensor_tensor(
                    out=scr2, in0=yt[:, h, :], scalar=1.0, in1=e2bc,
                    op0=ALU.mult, op1=ALU.mult, accum_out=fcols[h][:, 0:1])

            # ---------------- level 1: per-group attention pool ------------
            for b in range(n_blocks):
                # one 2 MB f32 HBM read, cast to bf16 on the way in
                fb = featp.tile([P, GB, KC, N], BF16)
                # two 1 MB halves: finer completion granularity halves the
                # PE's DMA-starve gaps at block boundaries (keeps HAM warm)
                for dh in range(2):
                    gh = GB // 2
                    nc.gpsimd.dma_start(
                        out=fb[:, dh * gh:(dh + 1) * gh, :, :],
                        in_=featT[b * GB + dh * gh:b * GB + (dh + 1) * gh,
                                  :, :].rearrange(
                            "g (c p) n -> p g c n", p=128))

                xt8 = xp.tile([P, GB, N], BF16)
                for j in range(GB):
                    xps = ps_x.tile([P, N], F32, tag="xps")
                    for c in range(KC):
                        nc.tensor.matmul(xps, hq_wT_sb[:, c, :],
                                         fb[:, j, c, :],
                                         start=(c == 0), stop=(c == KC - 1))
                    nc.scalar.activation(xt8[:, j, :], xps, AF.Tanh,
                                         bias=hq_b_sb)

                # scores for 4 groups on partitions {0,32,64,96} of one PSUM
                # tile (col-tiled M=32 matmuls), one exp per 4 groups
                sc4 = ps_s.tile([P, N], F32, tag="sc4")
                for j4 in range(4):
                    nc.tensor.matmul(sc4[32 * j4:32 * (j4 + 1), :],
                                     hk_w32, xt8[:, j4, :],
                                     start=True, stop=True,
                                     tile_position=(0, 32 * j4))
                e4 = ep.tile([P, N], BF16, tag="e4")
                zc4 = zp.tile([P, 1], F32, tag="zc4")
                nc.scalar.activation(e4, sc4, AF.Exp, accum_out=zc4[:, 0:1])
                zc4b = zp.tile([P, 1], BF16, tag="zc4b")
                nc.vector.tensor_copy(zc4b, zc4)
                zt4 = ps_t.tile([1, P], BF16, tag="zt4")
                nc.tensor.transpose(zt4, zc4b, ident128)
                g0 = b * GB
                nc.vector.tensor_copy(z_row[0:1, g0:g0 + 4],
                                      zt4[0:1, 0:128:32])
                for j4 in range(4):
                    g = b * GB + j4
                    ebc = ps_e.tile([P, N], F32, tag="ebc")
                    nc.tensor.matmul(
                        ebc, ones_bf[32 * j4:32 * j4 + 1, :],
                        e4[32 * j4:32 * j4 + 1, :],
                        start=True, stop=True,
                        tile_position=(32 * j4, 0))
                    prod = scratchp.tile([P, N], BF16, tag="prod")
                    nc.vector.scalar_tensor_tensor(
                        out=prod, in0=xt8[:, j4, :], scalar=1.0,
                        in1=ebc, op0=ALU.mult, op1=ALU.mult,
                        accum_out=uT_sb[:, g:g + 1])

                if b == n_blocks // 2 - 1:
                    start_half(0)
            start_half(1)

            # level-2 chains + combine: scheduled after all level-1 work
            with tc.high_priority(offset=-10_000_000):
                drain_half(0)
                drain_half(1)

            # ---------------- final softmax combine + output ---------------
            z2 = l2sb.tile([1, 1], F32)
            nc.vector.reduce_sum(z2, z2p, axis=AX.X)
            iz2 = l2sb.tile([1, 1], F32)
            nc.vector.reciprocal(iz2, z2)
            fsum = l2sb.tile([P, 1], F32)
            nc.vector.tensor_add(fsum, fcols[0], fcols[1])

            # out = (out_wT.T @ f_unnorm) * (1/Z2) + out_b
            iz2bc = ps_t.tile([P, 1], F32, tag="zt4", name="iz2bc")
            nc.tensor.matmul(iz2bc, ones_f32, iz2, start=True, stop=True)
            ops = ps_x.tile([P, 1], F32, tag="xps", name="ops")
            nc.tensor.matmul(ops, out_wT_sb, fsum, start=True, stop=True)
            out_sb = l2sb.tile([P, 1], F32)
            nc.vector.scalar_tensor_tensor(
                out=out_sb, in0=ops, scalar=iz2bc[:, 0:1], in1=out_b_sb,
                op0=ALU.mult, op1=ALU.add)
            nc.sync.dma_start(out=out[:, :], in_=out_sb)

    nc.compile()
    return nc


_NC_CACHE = None


def _get_nc():
    global _NC_CACHE
    if _NC_CACHE is None:
        _NC_CACHE = build_bass()
    return _NC_CACHE


def prep_in_maps(inputs: dict) -> list[dict]:
    feat = np.asarray(inputs["feat"], dtype=np.float32)
    # [G, N, IN] -> per-core [G_LOC, IN, N], contraction dim on partitions
    featT = np.ascontiguousarray(
        feat.reshape(N_CORES, G_LOC, N, IN_DIM).transpose(0, 1, 3, 2))

    def col(a):
        return np.ascontiguousarray(np.asarray(a, np.float32).reshape(-1, 1))

    shared = {
        "hq_wT": np.ascontiguousarray(np.asarray(inputs["hq_w"], np.float32).T),
        "hq_b": col(inputs["hq_b"]),
        "hk_w": col(inputs["hk_w"]),          # [1,128] -> [128,1]
        "mq_wT": np.ascontiguousarray(np.asarray(inputs["mq_w"], np.float32).T),
        "mq_b": col(inputs["mq_b"]),
        "mk_w": col(inputs["mk_w"]),
        "out_wT": np.ascontiguousarray(np.asarray(inputs["out_w"], np.float32).T),
        "out_b": col(inputs["out_b"]),
        "ident": np.eye(P, dtype=np.float32),
    }
    return [{"featT": featT[r], **shared} for r in range(N_CORES)]


def run_sharded(inputs: dict, trace: bool = False, tmpdir: str | None = None):
    """Returns (out [OUT_DIM] np.float32, BassKernelResults)."""
    nc = _get_nc()
    in_maps = prep_in_maps(inputs)
    res = run_bass_kernel_spmd(nc, in_maps, core_ids=list(range(N_CORES)),
                               trace=trace, tmpdir=tmpdir)
    out = np.asarray(res.results[0]["out"], dtype=np.float32).reshape(OUT_DIM)
    return out, res


def kernel(**inputs) -> np.ndarray:
    out, _ = run_sharded(inputs)
    return out


# revision 36
# speedup vs baseline: 1.0120x; 1.0120x over previous
"""Two-level additive attention pooling on 8 TRN2 NeuronCores.

Reference computation (G=1024 groups, N=512 set size, IN=256, H=128, O=128):
  x       = tanh(feat @ hq_w.T + hq_b)            [G,N,H]
  w1      = softmax(x @ hk_w.T + hk_b, axis=N)    (hk_b drops: softmax-invariant)
  stacked = sum_n w1 * x                          [G,H]
  y       = tanh(stacked @ mq_w.T + mq_b)         [G,H]
  w2      = softmax(y @ mk_w.T + mk_b, axis=G)    (mk_b drops)
  final   = sum_g w2 * y                          [H]
  out     = final @ out_w.T + out_b               [O]

Distribution: groups sharded 128/core across 8 cores (feat is 512 MB, the
dominant HBM traffic; the steady state is DMA-bound at ~330 GB/s/core).
Level-1 produces stackedT [H, 128] (f32) per core, AllGather'd in TWO halves
(32 KB/rank each) so the first half's level-2 matmul+tanh+exp+weighted-sum
runs overlapped with the second half of the feat stream; only the second
half's chain plus the tiny softmax combine sits in the kernel tail.

Level-1 layout ("layout A"): x kept transposed, xT [H=128 part, N=512 free];
feat is host-pre-transposed to [G, IN, N] so the contraction dim IN lands on
partitions.  feat streams from HBM as f32 (full memory-roofline traffic) and
is cast to bf16 in-flight by the SWDGE DMA — the PE runs bf16 matmuls (fp32
matmul measured ~4x slower).  Scores for 4 groups land on partitions
{0,32,64,96} of one PSUM tile via col-tiled M=32 matmuls so a single exp
covers 4 groups; exp's accum_out yields the softmax denominators for free.
The per-group weighted sum is one fused DVE scalar_tensor_tensor (multiply +
free-dim reduce) against a PE ones-broadcast of e.
"""

import numpy as np

import concourse.bass as bass
import concourse.bacc as bacc
import concourse.tile as tile
from concourse import mybir
from concourse.bass_utils import run_bass_kernel_spmd

F32 = mybir.dt.float32
BF16 = mybir.dt.bfloat16
AF = mybir.ActivationFunctionType
ALU = mybir.AluOpType
AX = mybir.AxisListType

N_CORES = 8
G, N, IN_DIM, HID, OUT_DIM = 1024, 512, 256, 128, 128
G_LOC = G // N_CORES          # 128 groups per core
KC = IN_DIM // 128            # 2 contraction chunks
GB = 4                        # groups per block (batched exp / DMA)
P = 128


def build_bass(g_loc: int = G_LOC) -> bacc.Bacc:
    G_LOC = g_loc  # noqa: N806 — local override for sim-sized builds
    n_blocks = G_LOC // GB
    HGL = G_LOC // 2              # groups per collective half
    nc = bacc.Bacc("TRN2", target_bir_lowering=False, debug=False,
                   num_devices=N_CORES)

    featT = nc.dram_tensor("featT", [G_LOC, IN_DIM, N], F32, kind="ExternalInput")
    hq_wT = nc.dram_tensor("hq_wT", [IN_DIM, HID], F32, kind="ExternalInput")
    hq_b = nc.dram_tensor("hq_b", [HID, 1], F32, kind="ExternalInput")
    hk_w = nc.dram_tensor("hk_w", [HID, 1], F32, kind="ExternalInput")
    mq_wT = nc.dram_tensor("mq_wT", [HID, HID], F32, kind="ExternalInput")
    mq_b = nc.dram_tensor("mq_b", [HID, 1], F32, kind="ExternalInput")
    mk_w = nc.dram_tensor("mk_w", [HID, 1], F32, kind="ExternalInput")
    out_wT = nc.dram_tensor("out_wT", [HID, OUT_DIM], F32, kind="ExternalInput")
    out_b = nc.dram_tensor("out_b", [OUT_DIM, 1], F32, kind="ExternalInput")
    ident = nc.dram_tensor("ident", [P, P], F32, kind="ExternalInput")
    out = nc.dram_tensor("out", [OUT_DIM, 1], F32, kind="ExternalOutput")

    rg = [list(range(N_CORES))]

    with tile.TileContext(nc) as tc:
        with (
            tc.tile_pool(name="consts", bufs=1) as consts,
            tc.tile_pool(name="accum", bufs=1) as accum,
            tc.tile_pool(name="dram", bufs=1, space="DRAM") as dram,
            tc.tile_pool(name="l2sb", bufs=1) as l2sb,
            tc.tile_pool(name="featp", bufs=8) as featp,
            tc.tile_pool(name="xp", bufs=3) as xp,
            tc.tile_pool(name="ep", bufs=3) as ep,
            tc.tile_pool(name="zp", bufs=3) as zp,
            tc.tile_pool(name="scratchp", bufs=3) as scratchp,
            tc.tile_pool(name="ps_x", bufs=3, space="PSUM") as ps_x,
            tc.tile_pool(name="ps_s", bufs=2, space="PSUM") as ps_s,
            tc.tile_pool(name="ps_e", bufs=2, space="PSUM") as ps_e,
            tc.tile_pool(name="ps_t", bufs=1, space="PSUM") as ps_t,
        ):
            # bf16 weight copies (cast in-flight by SWDGE)
            hq_wT_sb = consts.tile([P, KC, HID], BF16)
            for c in range(KC):
                nc.gpsimd.dma_start(out=hq_wT_sb[:, c, :],
                                    in_=hq_wT[c * 128:(c + 1) * 128, :])
            hk_w_sb = consts.tile([P, 1], BF16)
            nc.gpsimd.dma_start(out=hk_w_sb, in_=hk_w[:, :])
            mq_wT_sb = consts.tile([P, HID], F32)
            nc.sync.dma_start(out=mq_wT_sb, in_=mq_wT[:, :])
            mk_w_sb = consts.tile([P, 1], BF16)
            nc.gpsimd.dma_start(out=mk_w_sb, in_=mk_w[:, :])
            # f32 biases / final weights / identity (host-provided: building
            # it with gpsimd.affine_select would stall SWDGE feat DMA issue)
            hq_b_sb = consts.tile([P, 1], F32)
            nc.sync.dma_start(out=hq_b_sb, in_=hq_b[:, :])
            mq_b_sb = consts.tile([P, 1], F32)
            nc.sync.dma_start(out=mq_b_sb, in_=mq_b[:, :])
            out_wT_sb = consts.tile([P, OUT_DIM], F32)
            nc.sync.dma_start(out=out_wT_sb, in_=out_wT[:, :])
            out_b_sb = consts.tile([P, 1], F32)
            nc.sync.dma_start(out=out_b_sb, in_=out_b[:, :])
            ident128 = consts.tile([P, P], F32)
            nc.sync.dma_start(out=ident128, in_=ident[:, :])
            ones_bf = consts.tile([P, P], BF16)
            nc.vector.memset(ones_bf, 1.0)
            ones_f32 = consts.tile([1, P], F32)
            nc.vector.memset(ones_f32, 1.0)
            # hk_w replicated across 32 columns: the scores matmul runs M=32
            # (same cost as M=1, free-dim bound) so it fills a whole 32-row
            # PSUM strip — keeps the batched exp's input fully initialized.
            hk_w32 = consts.tile([P, 32], BF16)
            nc.vector.tensor_copy(hk_w32, hk_w_sb[:, 0:1].broadcast_to((P, 32)))

            uT_sb = accum.tile([P, G_LOC], F32)        # unnormalized stackedT
            z_row = accum.tile([1, G_LOC], F32)        # per-group softmax denom
            stackedT_sb = accum.tile([P, G_LOC], F32)  # normalized

            # level-2 state (filled per collective half)
            yt = l2sb.tile([P, 2, N_CORES * HGL], BF16)
            z2p = l2sb.tile([1, 2], F32)
            fcols = [l2sb.tile([P, 1], F32, tag=f"fcol{i}", name=f"fcol{i}")
                     for i in range(2)]
            cc_in = [dram.tile([P, HGL], F32, name=f"cc_in{h}", tag=f"cc_in{h}")
                     for h in range(2)]
            cc_out = [dram.tile([N_CORES * P, HGL], F32, addr_space="Shared",
                                name=f"cc_out{h}", tag=f"cc_out{h}")
                      for h in range(2)]

            def start_half(h):
                """Normalize half h of stackedT and trigger its AllGather
                (cheap; runs inline with the feat stream)."""
                lo, hi = h * HGL, (h + 1) * HGL
                inv_z = zp.tile([1, HGL], F32, tag="inv_z", name="inv_z")
                nc.vector.reciprocal(inv_z, z_row[0:1, lo:hi])
                izbc = ps_e.tile([P, HGL], F32, tag="ebc", name="izbc")
                nc.tensor.matmul(izbc, ones_f32, inv_z, start=True, stop=True)
                nc.vector.tensor_mul(stackedT_sb[:, lo:hi],
                                     uT_sb[:, lo:hi], izbc)
                nc.sync.dma_start(out=cc_in[h][:, :],
                                  in_=stackedT_sb[:, lo:hi])
                nc.gpsimd.collective_compute(
                    "AllGather", ALU.bypass, replica_groups=rg,
                    ins=[cc_in[h][:, :].opt()], outs=[cc_out[h][:, :].opt()])

            def drain_half(h):
                """Consume half h's gathered stackedT (level-2 chain).  Runs
                under low priority so its collective-wait never sits in front
                of level-1 work in the in-order engine queues."""
                st_h = l2sb.tile([P, N_CORES, HGL], F32, tag=f"st{h}",
                                 name=f"st{h}")
                nc.sync.dma_start(
                    out=st_h,
                    in_=cc_out[h][:, :].rearrange("(r p) g -> p r g", p=128))
                CW = N_CORES * HGL
                yps = ps_x.tile([P, CW], F32, tag="xps", name="yps")
                for r in range(N_CORES):
                    nc.tensor.matmul(yps[:, r * HGL:(r + 1) * HGL],
                                     mq_wT_sb, st_h[:, r, :],
                                     start=True, stop=True)
                nc.scalar.activation(yt[:, h, :], yps, AF.Tanh, bias=mq_b_sb)
                s2ps = ps_t.tile([1, CW], F32, tag="zt4", name="s2ps")
                nc.tensor.matmul(s2ps, mk_w_sb, yt[:, h, :],
                                 start=True, stop=True)
                e2 = ep.tile([1, CW], BF16, tag="e2", name="e2")
                nc.scalar.activation(e2, s2ps, AF.Exp,
                                     accum_out=z2p[0:1, h:h + 1])
                e2bc = ps_e.tile([P, CW], F32, tag="ebc", name="e2bc")
                nc.tensor.matmul(e2bc, ones_bf[0:1, :], e2,
                                 start=True, stop=True)
                scr2 = l2sb.tile([P, CW], BF16, tag=f"scr2_{h}",
                                 name=f"scr2_{h}")
                nc.vector.scalar_tensor_tensor(
                    out=scr2, in0=yt[:, h, :], scalar=1.0, in1=e2bc,
                    op0=ALU.mult, op1=ALU.mult, accum_out=fcols[h][:, 0:1])

            # ---------------- level 1: per-group attention pool ------------
            for b in range(n_blocks):
                # one 2 MB f32 HBM read, cast to bf16 on the way in
                fb = featp.tile([P, GB, KC, N], BF16)
                # two 1 MB halves: finer completion granularity halves the
                # PE's DMA-starve gaps at block boundaries (keeps HAM warm)
                for dh in range(2):
                    gh = GB // 2
                    nc.gpsimd.dma_start(
                        out=fb[:, dh * gh:(dh + 1) * gh, :, :],
                        in_=featT[b * GB + dh * gh:b * GB + (dh + 1) * gh,
                                  :, :].rearrange(
                            "g (c p) n -> p g c n", p=128))

                xt8 = xp.tile([P, GB, N], BF16)
                for j in range(GB):
                    xps = ps_x.tile([P, N], F32, tag="xps")
                    for c in range(KC):
                        nc.tensor.matmul(xps, hq_wT_sb[:, c, :],
                                         fb[:, j, c, :],
                                         start=(c == 0), stop=(c == KC - 1))
                    nc.scalar.activation(xt8[:, j, :], xps, AF.Tanh,
                                         bias=hq_b_sb)

                # scores for 4 groups on partitions {0,32,64,96} of one PSUM
                # tile (col-tiled M=32 matmuls), one exp per 4 groups
                sc4 = ps_s.tile([P, N], F32, tag="sc4")
                for j4 in range(4):
                    nc.tensor.matmul(sc4[32 * j4:32 * (j4 + 1), :],
                                     hk_w32, xt8[:, j4, :],
                                     start=True, stop=True,
                                     tile_position=(0, 32 * j4))
                e4 = ep.tile([P, N], BF16, tag="e4")
                zc4 = zp.tile([P, 1], F32, tag="zc4")
                nc.scalar.activation(e4, sc4, AF.Exp, accum_out=zc4[:, 0:1])
                zt4 = ps_t.tile([1, P], F32, tag="zt4")
                nc.tensor.transpose(zt4, zc4, ident128)
                g0 = b * GB
                nc.vector.tensor_copy(z_row[0:1, g0:g0 + 4],
                                      zt4[0:1, 0:128:32])
                for j4 in range(4):
                    g = b * GB + j4
                    ebc = ps_e.tile([P, N], F32, tag="ebc")
                    nc.tensor.matmul(
                        ebc, ones_bf[32 * j4:32 * j4 + 1, :],
                        e4[32 * j4:32 * j4 + 1, :],
                        start=True, stop=True,
                        tile_position=(32 * j4, 0))
                    prod = scratchp.tile([P, N], BF16, tag="prod")
                    nc.vector.scalar_tensor_tensor(
                        out=prod, in0=xt8[:, j4, :], scalar=1.0,
                        in1=ebc, op0=ALU.mult, op1=ALU.mult,
                        accum_out=uT_sb[:, g:g + 1])

                if b == n_blocks // 2 - 1:
                    start_half(0)
            start_half(1)

            # level-2 chains + combine: scheduled after all level-1 work
            with tc.high_priority(offset=-10_000_000):
                drain_half(0)
                drain_half(1)

            # ---------------- final softmax combine + output ---------------
            z2 = l2sb.tile([1, 1], F32)
            nc.vector.reduce_sum(z2, z2p, axis=AX.X)
            iz2 = l2sb.tile([1, 1], F32)
            nc.vector.reciprocal(iz2, z2)
            fsum = l2sb.tile([P, 1], F32)
            nc.vector.tensor_add(fsum, fcols[0], fcols[1])

            # out = (out_wT.T @ f_unnorm) * (1/Z2) + out_b
            iz2bc = ps_t.tile([P, 1], F32, tag="zt4", name="iz2bc")
            nc.tensor.matmul(iz2bc, ones_f32, iz2, start=True, stop=True)
            ops = ps_x.tile([P, 1], F32, tag="xps", name="ops")
            nc.tensor.matmul(ops, out_wT_sb, fsum, start=True, stop=True)
            out_sb = l2sb.tile([P, 1], F32)
            nc.vector.scalar_tensor_tensor(
                out=out_sb, in0=ops, scalar=iz2bc[:, 0:1], in1=out_b_sb,
                op0=ALU.mult, op1=ALU.add)
            nc.sync.dma_start(out=out[:, :], in_=out_sb)

    nc.compile()
    return nc


_NC_CACHE = None


def _get_nc():
    global _NC_CACHE
    if _NC_CACHE is None:
        _NC_CACHE = build_bass()
    return _NC_CACHE


def prep_in_maps(inputs: dict) -> list[dict]:
    feat = np.asarray(inputs["feat"], dtype=np.float32)
    # [G, N, IN] -> per-core [G_LOC, IN, N], contraction dim on partitions
    featT = np.ascontiguousarray(
        feat.reshape(N_CORES, G_LOC, N, IN_DIM).transpose(0, 1, 3, 2))

    def col(a):
        return np.ascontiguousarray(np.asarray(a, np.float32).reshape(-1, 1))

    shared = {
        "hq_wT": np.ascontiguousarray(np.asarray(inputs["hq_w"], np.float32).T),
        "hq_b": col(inputs["hq_b"]),
        "hk_w": col(inputs["hk_w"]),          # [1,128] -> [128,1]
        "mq_wT": np.ascontiguousarray(np.asarray(inputs["mq_w"], np.float32).T),
        "mq_b": col(inputs["mq_b"]),
        "mk_w": col(inputs["mk_w"]),
        "out_wT": np.ascontiguousarray(np.asarray(inputs["out_w"], np.float32).T),
        "out_b": col(inputs["out_b"]),
        "ident": np.eye(P, dtype=np.float32),
    }
    return [{"featT": featT[r], **shared} for r in range(N_CORES)]


def run_sharded(inputs: dict, trace: bool = False, tmpdir: str | None = None):
    """Returns (out [OUT_DIM] np.float32, BassKernelResults)."""
    nc = _get_nc()
    in_maps = prep_in_maps(inputs)
    res = run_bass_kernel_spmd(nc, in_maps, core_ids=list(range(N_CORES)),
                               trace=trace, tmpdir=tmpdir)
    out = np.asarray(res.results[0]["out"], dtype=np.float32).reshape(OUT_DIM)
    return out, res


def kernel(**inputs) -> np.ndarray:
    out, _ = run_sharded(inputs)
    return out
